# revision 21
# baseline (speedup 1.0000x reference)
"""Att2in2Core fused kernel for 8 Trainium2 NeuronCores.

Sharding: attention part data-parallel over batch (32 rows/core); the
LSTM-with-maxout part tensor-parallel over H (256 cols/core spanning all
five i2h/h2h gate groups), bridged by two bf16 AllGathers of att_res.

Self-contained: hardcodes B=256, S=49, H=2048 and the sharding. The
only runtime-variant input handling is `time` (captions column select +
state zeroing at t==0), done host-side as input preparation.
"""
import os
import sys

os.environ.setdefault("NEURON_RT_DBG_RDH_CC", "0")
sys.path.insert(0, "/opt/trn_rl_repo")

import numpy as np
import ml_dtypes

import concourse.bass as bass
import concourse.bacc as bacc
import concourse.mybir as mybir
import concourse.tile as tile
from concourse.bass_utils import run_bass_kernel_spmd

BF16 = ml_dtypes.bfloat16
B, S, H = 256, 49, 2048
NCORE = 8
BC = B // NCORE            # 32 batch rows per core
HC = H // NCORE            # 256 H cols per core (LSTM shard)
R = S * BC                 # 1568 attention rows per core (b-major)
KT = H // 128              # 16 contraction tiles
AT = H // 128              # 16 output a-tiles
# r-tiles (b-major, r = b*49 + s): (roff, rsz, b_off, b_cnt)
RT = [(0, 490, 0, 10), (490, 490, 10, 10), (980, 490, 20, 10), (1470, 98, 30, 2)]
F32 = mybir.dt.float32
BF = mybir.dt.bfloat16
AF = mybir.ActivationFunctionType
ALU = mybir.AluOpType

_NC_CACHE = {}


def build_nc():
    nc = bacc.Bacc("TRN2", target_bir_lowering=False, debug=False,
                   num_devices=NCORE)

    def inp(name, shape, dt=BF):
        return nc.declare_dram_parameter(name, list(shape), dt, isOutput=False)

    vt_d = inp("vt", (H, R))                      # V shard ^T, b-major cols
    attw_d = inp("attw", (AT, 128, KT, 128))      # pre-tiled [m][p][k][j]
    ctxw_d = inp("ctxw", (AT, 128, KT, 128))
    h2attw_d = inp("h2attw", (H, H))
    i2h_d = inp("i2h", (H + 1, 5 * HC))           # shard ^T + bias row
    h2h_d = inp("h2h", (H, 5 * HC))               # shard ^T
    a2c_d = inp("a2c", (H + 1, 2 * HC))           # shard ^T + bias row
    embt_d = inp("embt", (H, B))                  # emb[it] ^T (pre-relu)
    hprevt_d = inp("hprevt", (H, B))              # h_prev ^T
    hprevown_d = inp("hprevown", (H, BC))         # own batch slice of ^T
    cprev_d = inp("cprev", (B, HC), F32)          # c_prev[:, own H chunk]
    attb_d = inp("attb", (H,), F32)               # att_b
    tanhb_d = inp("tanhb", (H,), F32)             # ctx_b + h2att_b
    alpha_d = inp("alpha", (H,))                  # alpha_W[0]

    outh_d = nc.declare_dram_parameter("out_h", [B, HC], F32, isOutput=True)
    outc_d = nc.declare_dram_parameter("out_c", [B, HC], F32, isOutput=True)

    with tile.TileContext(nc) as tc:
        with (
            tc.tile_pool(name="sb", bufs=1) as sb,
            tc.tile_pool(name="ps", bufs=1, space="PSUM") as ps,
            tc.tile_pool(name="dram", bufs=1, space="DRAM") as dram,
        ):
            # ---- phase B: mm1  att_feats^T = relu(attW @ V^T + b) -----
            # emission order = DMA priority: first weights/Vt for mm1.
            attb_sb = sb.tile([128, AT], F32)
            nc.sync.dma_start(out=attb_sb[:],
                              in_=attb_d.ap().rearrange("(t p) -> p t", p=128))
            wa0 = sb.tile([128, KT, 128], BF, tag="wtile", bufs=3, name="wa0")
            nc.gpsimd.dma_start(out=wa0[:], in_=attw_d.ap()[0])
            # tiny dummy AllGather: pays the cold ncfw/trigger latency early,
            # hidden under mm1, so the real gathers start fast
            warm_in = dram.tile([1, 32], BF)
            warm_out = dram.tile([NCORE, 32], BF, addr_space="Shared")
            warm_sb = sb.tile([1, 32], BF)
            nc.vector.memset(warm_sb[:], 0.0)
            nc.gpsimd.dma_start(out=warm_in[:], in_=warm_sb[:])
            nc.gpsimd.collective_compute(
                "AllGather", ALU.bypass,
                replica_groups=[list(range(NCORE))],
                ins=[warm_in.opt()], outs=[warm_out.opt()])
            vt_sb = sb.tile([128, KT, R], BF)
            vt_v = vt_d.ap().rearrange("(k p) r -> p k r", p=128)
            for k in range(KT):
                eng = nc.sync if k % 2 == 0 else nc.scalar
                eng.dma_start(out=vt_sb[:, k, :], in_=vt_v[:, k, :])
            a2c_sb = sb.tile([128, KT, 2 * HC], BF)
            nc.sync.dma_start(
                out=a2c_sb[:],
                in_=a2c_d.ap()[0:H, :].rearrange("(k p) n -> p k n", p=128))
            att_f = sb.tile([128, AT, R], BF)  # att_feats^T, bf16

            for m in range(AT):
                if m == 0:
                    wa = wa0
                else:
                    wa = sb.tile([128, KT, 128], BF, tag="wtile", bufs=3,
                                 name=f"wa{m}")
                    nc.gpsimd.dma_start(out=wa[:], in_=attw_d.ap()[m])
                psbs = [ps.tile([128, rsz], F32, tag="acc", bufs=7,
                                name=f"ps_mm1_{m}_{roff}")
                        for (roff, rsz, bo, bt) in RT]
                for k in range(KT):
                    for ni, (roff, rsz, bo, bt) in enumerate(RT):
                        nc.tensor.matmul(psbs[ni][:], wa[:, k],
                                         vt_sb[:, k, roff:roff + rsz],
                                         start=(k == 0), stop=(k == KT - 1))
                for ni, (roff, rsz, bo, bt) in enumerate(RT):
                    nc.scalar.activation(att_f[:, m, roff:roff + rsz], psbs[ni][:],
                                         AF.Relu, bias=attb_sb[:, m:m + 1])

            # ---- phase A: att_h (b-stationary) + PE transpose ---------
            hprevown_sb = sb.tile([128, KT, BC], BF)
            nc.sync.dma_start(out=hprevown_sb[:],
                              in_=hprevown_d.ap().rearrange("(k p) b -> p k b", p=128))
            from concourse import masks
            ident32 = sb.tile([32, 32], BF)
            masks.make_identity(nc, ident32[:])
            h2att_v = h2attw_d.ap().rearrange("(k p) n -> p k n", p=128)
            ah_sbA = sb.tile([32, H], BF)
            for n in range(4):
                psa = ps.tile([32, 512], F32, tag="acc", bufs=7, name=f"ps_ah_{n}")
                for k in range(KT):
                    whn = sb.tile([128, 512], BF, tag="whtile", bufs=4,
                                  name=f"wh{n}_{k}")
                    nc.gpsimd.dma_start(out=whn[:],
                                        in_=h2att_v[:, k, n * 512:(n + 1) * 512])
                    nc.tensor.matmul(psa[:], hprevown_sb[:, k, :], whn[:],
                                     start=(k == 0), stop=(k == KT - 1))
                nc.vector.tensor_copy(ah_sbA[:, n * 512:(n + 1) * 512], psa[:])
            atthT = sb.tile([128, AT, BC, 1], F32)
            for m in range(AT):
                pst = ps.tile([128, BC], BF, tag="acc", bufs=7, name=f"ps_at_{m}")
                nc.tensor.transpose(pst[:], ah_sbA[:, m * 128:(m + 1) * 128],
                                    ident32[:])
                nc.vector.tensor_copy(atthT[:, m, :, 0], pst[:])

            # ---- phase C: mm2 + tanh + scores -------------------------
            tanhb_sb = sb.tile([128, AT], F32)
            nc.sync.dma_start(out=tanhb_sb[:],
                              in_=tanhb_d.ap().rearrange("(t p) -> p t", p=128))
            alpha_sb = sb.tile([128, AT], BF)
            nc.sync.dma_start(out=alpha_sb[:],
                              in_=alpha_d.ap().rearrange("(t p) -> p t", p=128))
            # all 4 r-tile score accumulators share ONE psum bank at
            # partitions 0/32/64/96 (M=1 col-group matmuls)
            scb = ps.tile([97, 512], F32, tag="sc", bufs=1)
            for m2 in range(AT):
                wc = sb.tile([128, KT, 128], BF, tag="wtile", bufs=3, name=f"wc{m2}")
                nc.sync.dma_start(out=wc[:], in_=ctxw_d.ap()[m2])
                ps2s = [ps.tile([128, rsz], F32, tag="acc", bufs=7,
                                name=f"ps_mm2_{m2}_{roff}")
                        for (roff, rsz, bo, bt) in RT]
                for k in range(KT):
                    for ni, (roff, rsz, bo, bt) in enumerate(RT):
                        nc.tensor.matmul(ps2s[ni][:], wc[:, k],
                                         att_f[:, k, roff:roff + rsz],
                                         start=(k == 0), stop=(k == KT - 1))
                for ni, (roff, rsz, bo, bt) in enumerate(RT):
                    ps2 = ps2s[ni]
                    nc.vector.tensor_tensor(
                        out=ps2.rearrange("p (b s) -> p b s", s=S),
                        in0=ps2.rearrange("p (b s) -> p b s", s=S),
                        in1=atthT[:, m2, bo:bo + bt, :].broadcast_to((128, bt, S)),
                        op=ALU.add)
                    dot_sb = sb.tile([128, rsz], BF, tag="dot", bufs=3,
                                     name=f"dot{m2}_{roff}")
                    nc.scalar.activation(dot_sb[:], ps2[:], AF.Tanh,
                                         bias=tanhb_sb[:, m2:m2 + 1])
                    nc.tensor.matmul(scb[32 * ni:32 * ni + 1, 0:rsz],
                                     alpha_sb[:, m2:m2 + 1], dot_sb[:],
                                     start=(m2 == 0), stop=(m2 == AT - 1),
                                     tile_position=(0, 32 * ni))

            # ---- phase D: softmax over s (per b) ----------------------
            scores_sb = sb.tile([1, R], BF)
            for ni, (roff, rsz, bo, bt) in enumerate(RT):
                nc.scalar.activation(scores_sb[:, roff:roff + rsz],
                                     scb[32 * ni:32 * ni + 1, 0:rsz], AF.Exp)
            ssum = sb.tile([1, BC], F32)
            nc.vector.tensor_reduce(ssum[:],
                                    scores_sb.rearrange("p (b s) -> p b s", s=S),
                                    axis=mybir.AxisListType.X, op=ALU.add)
            rinv = sb.tile([1, BC, 1], F32)
            nc.vector.reciprocal(rinv[:, :, 0], ssum[:])
            w_bf = sb.tile([1, R], BF)
            nc.vector.tensor_tensor(
                out=w_bf.rearrange("p (b s) -> p b s", s=S),
                in0=scores_sb.rearrange("p (b s) -> p b s", s=S),
                in1=rinv.broadcast_to((1, BC, S)),
                op=ALU.mult)
            w_row = dram.tile([1, R], BF)
            nc.gpsimd.dma_start(out=w_row[:], in_=w_bf[:])
            w_exp = sb.tile([128, R], BF)
            nc.gpsimd.dma_start(out=w_exp[:], in_=w_row.broadcast_to((128, R)))

            # ---- phase E: att_res^T + 2 half AllGathers ---------------
            # ar_all[p, m, b]; DRAM halves laid out [p][kk*b] so both the
            # store and the post-gather reload use >=512B contiguous runs.
            ar_half = [sb.tile([128, AT // 2, BC], BF, name=f"ar_half{h}")
                       for h in range(2)]
            for m in range(AT):
                prodm = sb.tile([128, R], BF, tag="prod", bufs=1, name=f"prod{m}")
                nc.vector.tensor_tensor(out=prodm[:], in0=att_f[:, m, :],
                                        in1=w_exp[:], op=ALU.mult)
                arf = sb.tile([128, BC], F32, tag="arf", bufs=2, name=f"arf{m}")
                nc.vector.tensor_reduce(arf[:],
                                        prodm.rearrange("p (b s) -> p b s", s=S),
                                        axis=mybir.AxisListType.X, op=ALU.add)
                nc.scalar.activation(ar_half[m // 8][:, m % 8, :], arf[:], AF.Copy)
            ar_loc = [dram.tile([128, KT // 2 * BC], BF, name=f"ar_loc{h}")
                      for h in range(2)]
            ar_g = [dram.tile([NCORE * 128, KT // 2 * BC], BF,
                              addr_space="Shared", name=f"ar_g{h}")
                    for h in range(2)]
            arT = [sb.tile([128, KT // 2, NCORE, BC], BF, name=f"arT{h}")
                   for h in range(2)]
            for h in range(2):
                nc.gpsimd.dma_start(
                    out=ar_loc[h].rearrange("p (kk b) -> p kk b", b=BC),
                    in_=ar_half[h][:])
                nc.gpsimd.collective_compute(
                    "AllGather", ALU.bypass,
                    replica_groups=[list(range(NCORE))],
                    ins=[ar_loc[h].opt()], outs=[ar_g[h].opt()])
                for c in range(NCORE):
                    eng = nc.scalar if c % 2 == 0 else nc.sync
                    eng.dma_start(
                        out=arT[h][:, :, c, :],
                        in_=ar_g[h][c * 128:(c + 1) * 128, :].rearrange(
                            "p (kk b) -> p kk b", b=BC))

            # ---- phase F: LSTM (tensor-parallel over H chunk) ---------
            xt_sb = sb.tile([128, KT, B], BF)
            nc.sync.dma_start(out=xt_sb[:],
                              in_=embt_d.ap().rearrange("(k p) b -> p k b", p=128))
            nc.scalar.activation(xt_sb[:], xt_sb[:], AF.Relu)
            hprev_sb = sb.tile([128, KT, B], BF)
            nc.sync.dma_start(out=hprev_sb[:],
                              in_=hprevt_d.ap().rearrange("(k p) b -> p k b", p=128))
            cprev_sb = sb.tile([128, 2, HC], F32)
            nc.sync.dma_start(out=cprev_sb[:],
                              in_=cprev_d.ap().rearrange("(m p) h -> p m h", p=128))
            ones_sb = sb.tile([1, B], BF)
            nc.vector.memset(ones_sb[:], 1.0)
            i2hb_sb = sb.tile([1, 5 * HC], BF)
            nc.sync.dma_start(out=i2hb_sb[:], in_=i2h_d.ap()[H:H + 1, :])
            a2cb_sb = sb.tile([1, 2 * HC], BF)
            nc.sync.dma_start(out=a2cb_sb[:], in_=a2c_d.ap()[H:H + 1, :])
            i2h_v = i2h_d.ap()[0:H, :].rearrange("(k p) n -> p k n", p=128)
            h2h_v = h2h_d.ap().rearrange("(k p) n -> p k n", p=128)

            LT = [(0, 512), (512, 512), (1024, 256)]
            psl = {}
            sig_if, sig_o, intr_a = {}, {}, {}
            for (noff, nsz) in LT:
                for mb in range(2):
                    psl[(noff, mb)] = ps.tile([128, nsz], F32, tag="acc", bufs=7,
                                              name=f"ps_l{noff}_{mb}")
                for k in range(KT):
                    ri = sb.tile([128, nsz], BF, tag="lrhs", bufs=4,
                                 name=f"ri{noff}_{k}")
                    nc.sync.dma_start(out=ri[:], in_=i2h_v[:, k, noff:noff + nsz])
                    for mb in range(2):
                        nc.tensor.matmul(psl[(noff, mb)][:],
                                         xt_sb[:, k, mb * 128:(mb + 1) * 128],
                                         ri[:], start=(k == 0), stop=False)
                for k in range(KT):
                    rh = sb.tile([128, nsz], BF, tag="lrhs", bufs=4,
                                 name=f"rh{noff}_{k}")
                    nc.sync.dma_start(out=rh[:], in_=h2h_v[:, k, noff:noff + nsz])
                    for mb in range(2):
                        nc.tensor.matmul(psl[(noff, mb)][:],
                                         hprev_sb[:, k, mb * 128:(mb + 1) * 128],
                                         rh[:], start=False, stop=False)
                for mb in range(2):
                    nc.tensor.matmul(psl[(noff, mb)][:],
                                     ones_sb[:, mb * 128:(mb + 1) * 128],
                                     i2hb_sb[:, noff:noff + nsz],
                                     start=False, stop=(noff == 0))
                if noff >= 512:
                    # a2c contribution lands in in_tr cols [768:1280) of 5*HC
                    if noff == 512:
                        dsl, acs = slice(256, 512), slice(0, 256)
                    else:
                        dsl, acs = slice(0, 256), slice(256, 512)
                    for k in range(KT):
                        h, kk = divmod(k, KT // 2)
                        for mb in range(2):
                            nc.tensor.matmul(psl[(noff, mb)][:, dsl],
                                             arT[h][:, kk, mb * 4:(mb + 1) * 4, :],
                                             a2c_sb[:, k, acs],
                                             start=False, stop=False)
                    for mb in range(2):
                        nc.tensor.matmul(psl[(noff, mb)][:, dsl],
                                         ones_sb[:, mb * 128:(mb + 1) * 128],
                                         a2cb_sb[:, acs], start=False, stop=True)
                # drain
                for mb in range(2):
                    p_ = psl[(noff, mb)]
                    if noff == 0:
                        t_ = sb.tile([128, 512], F32, tag="sif", bufs=2,
                                     name=f"sif{mb}")
                        nc.scalar.activation(t_[:], p_[:], AF.Sigmoid)
                        sig_if[mb] = t_
                    elif noff == 512:
                        t_ = sb.tile([128, 256], F32, tag="sio", bufs=2,
                                     name=f"sio{mb}")
                        nc.scalar.activation(t_[:], p_[:, 0:256], AF.Sigmoid)
                        sig_o[mb] = t_
                        t2_ = sb.tile([128, 256], F32, tag="itra", bufs=2,
                                      name=f"itra{mb}")
                        nc.vector.tensor_copy(t2_[:], p_[:, 256:512])
                        intr_a[mb] = t2_

            for mb in range(2):
                bsl = slice(mb * 128, (mb + 1) * 128)
                in_tr = sb.tile([128, 256], F32, tag="intr", bufs=2, name=f"intr{mb}")
                nc.vector.tensor_tensor(out=in_tr[:],
                                        in0=intr_a[mb][:],
                                        in1=psl[(1024, mb)][:, 0:256], op=ALU.max)
                t_fc = sb.tile([128, 256], F32, tag="tfc", bufs=2, name=f"tfc{mb}")
                nc.vector.tensor_tensor(out=t_fc[:], in0=sig_if[mb][:, 256:512],
                                        in1=cprev_sb[:, mb, :], op=ALU.mult)
                t_ii = sb.tile([128, 256], F32, tag="tii", bufs=2, name=f"tii{mb}")
                nc.vector.tensor_tensor(out=t_ii[:], in0=sig_if[mb][:, 0:256],
                                        in1=in_tr[:], op=ALU.mult)
                next_c = sb.tile([128, 256], F32, tag="nc", bufs=2, name=f"nc{mb}")
                nc.vector.tensor_tensor(out=next_c[:], in0=t_fc[:], in1=t_ii[:],
                                        op=ALU.add)
                nc.gpsimd.dma_start(out=outc_d.ap()[bsl, :], in_=next_c[:])
                tanh_c = sb.tile([128, 256], F32, tag="tfc", bufs=2, name=f"tc{mb}")
                nc.scalar.activation(tanh_c[:], next_c[:], AF.Tanh)
                next_h = sb.tile([128, 256], F32, tag="tii", bufs=2, name=f"nh{mb}")
                nc.vector.tensor_tensor(out=next_h[:], in0=sig_o[mb][:],
                                        in1=tanh_c[:], op=ALU.mult)
                nc.gpsimd.dma_start(out=outh_d.ap()[bsl, :], in_=next_h[:])

    nc.compile()
    return nc


def _tile4(wt):
    """(2048,2048) [f,out] -> (16 m,128 p,16 k,128 j) contiguous bf16."""
    return np.ascontiguousarray(
        wt.reshape(KT, 128, AT, 128).transpose(2, 1, 0, 3)).astype(BF16)


def _bf(a):
    return np.ascontiguousarray(a).astype(BF16)


def prepare_inputs(V, v_g, state, fc_W, fc_b, att_W, att_b, emb, ctx_W, ctx_b,
                   a2c_W, a2c_b, i2h_W, i2h_b, h2h_W, h2h_b,
                   h2att_W, h2att_b, alpha_W, alpha_b, captions, time):
    t = int(np.asarray(time))
    V = np.asarray(V, np.float32)
    state = np.asarray(state, np.float32)
    if t == 0:
        state = np.zeros_like(state)
    h_prev = state[0, -1]
    c_prev = state[1, -1]
    it = np.asarray(captions, np.int64)[:, t]
    emb_it = np.asarray(emb, np.float32)[it]          # (B, H)

    attw4 = _tile4(np.asarray(att_W, np.float32).T)
    ctxw4 = _tile4(np.asarray(ctx_W, np.float32).T)
    h2attw4 = _bf(np.asarray(h2att_W, np.float32).T)
    embt = _bf(emb_it.T)
    hprevt = _bf(h_prev.T)
    attb = np.ascontiguousarray(att_b, np.float32)
    tanhb = np.ascontiguousarray(np.asarray(ctx_b, np.float32)
                                 + np.asarray(h2att_b, np.float32))
    alpha = _bf(np.asarray(alpha_W, np.float32)[0])
    ib = np.asarray(i2h_b, np.float32) + np.asarray(h2h_b, np.float32)
    i2h_W = np.asarray(i2h_W, np.float32)
    h2h_W = np.asarray(h2h_W, np.float32)
    a2c_W = np.asarray(a2c_W, np.float32)
    a2c_b = np.asarray(a2c_b, np.float32)

    in_maps = []
    for c in range(NCORE):
        bs = slice(c * BC, (c + 1) * BC)
        hs = [slice(g * H + c * HC, g * H + (c + 1) * HC) for g in range(5)]
        i2h_blk = np.concatenate([i2h_W[s] for s in hs], 0)       # (1280, H)
        h2h_blk = np.concatenate([h2h_W[s] for s in hs], 0)
        ib_blk = np.concatenate([ib[s] for s in hs], 0)           # (1280,)
        a2c_blk = np.concatenate([a2c_W[s] for s in hs[:2]], 0)   # (512, H)
        a2cb_blk = np.concatenate([a2c_b[s] for s in hs[:2]], 0)
        vt = _bf(V[bs].transpose(2, 0, 1).reshape(H, R))
        in_maps.append({
            "vt": vt,
            "attw": attw4, "ctxw": ctxw4, "h2attw": h2attw4,
            "i2h": _bf(np.vstack([i2h_blk.T, ib_blk[None, :]])),
            "h2h": _bf(h2h_blk.T),
            "a2c": _bf(np.vstack([a2c_blk.T, a2cb_blk[None, :]])),
            "embt": embt, "hprevt": hprevt,
            "hprevown": np.ascontiguousarray(hprevt[:, bs]),
            "cprev": np.ascontiguousarray(c_prev[:, c * HC:(c + 1) * HC],
                                          ).astype(np.float32),
            "attb": attb, "tanhb": tanhb, "alpha": alpha,
        })
    return in_maps


def kernel(**inputs):
    in_maps = prepare_inputs(**inputs)
    if "nc" not in _NC_CACHE:
        _NC_CACHE["nc"] = build_nc()
    nc = _NC_CACHE["nc"]
    res = run_bass_kernel_spmd(nc, in_maps, core_ids=list(range(NCORE)))
    next_h = np.concatenate([res.results[c]["out_h"] for c in range(NCORE)], 1)
    next_c = np.concatenate([res.results[c]["out_c"] for c in range(NCORE)], 1)
    next_h = np.asarray(next_h, np.float32)
    next_c = np.asarray(next_c, np.float32)
    return next_h[:, None, :], next_h[None], next_c[None]


# revision 22
# speedup vs baseline: 1.0095x; 1.0095x over previous
"""Att2in2Core fused kernel for 8 Trainium2 NeuronCores.

Sharding: attention part data-parallel over batch (32 rows/core); the
LSTM-with-maxout part tensor-parallel over H (256 cols/core spanning all
five i2h/h2h gate groups), bridged by two bf16 AllGathers of att_res.

Self-contained: hardcodes B=256, S=49, H=2048 and the sharding. The
only runtime-variant input handling is `time` (captions column select +
state zeroing at t==0), done host-side as input preparation.
"""
import os
import sys

os.environ.setdefault("NEURON_RT_DBG_RDH_CC", "0")
sys.path.insert(0, "/opt/trn_rl_repo")

import numpy as np
import ml_dtypes

import concourse.bass as bass
import concourse.bacc as bacc
import concourse.mybir as mybir
import concourse.tile as tile
from concourse.bass_utils import run_bass_kernel_spmd

BF16 = ml_dtypes.bfloat16
B, S, H = 256, 49, 2048
NCORE = 8
BC = B // NCORE            # 32 batch rows per core
HC = H // NCORE            # 256 H cols per core (LSTM shard)
R = S * BC                 # 1568 attention rows per core (b-major)
KT = H // 128              # 16 contraction tiles
AT = H // 128              # 16 output a-tiles
# r-tiles (b-major, r = b*49 + s): (roff, rsz, b_off, b_cnt)
RT = [(0, 490, 0, 10), (490, 490, 10, 10), (980, 490, 20, 10), (1470, 98, 30, 2)]
F32 = mybir.dt.float32
BF = mybir.dt.bfloat16
AF = mybir.ActivationFunctionType
ALU = mybir.AluOpType

_NC_CACHE = {}


def build_nc():
    nc = bacc.Bacc("TRN2", target_bir_lowering=False, debug=False,
                   num_devices=NCORE)

    def inp(name, shape, dt=BF):
        return nc.declare_dram_parameter(name, list(shape), dt, isOutput=False)

    vt_d = inp("vt", (H, R))                      # V shard ^T, b-major cols
    attw_d = inp("attw", (AT, 128, KT, 128))      # pre-tiled [m][p][k][j]
    ctxw_d = inp("ctxw", (AT, 128, KT, 128))
    h2attw_d = inp("h2attw", (H, H))
    i2h_d = inp("i2h", (H + 1, 5 * HC))           # shard ^T + bias row
    h2h_d = inp("h2h", (H, 5 * HC))               # shard ^T
    a2c_d = inp("a2c", (H + 1, 2 * HC))           # shard ^T + bias row
    embt_d = inp("embt", (H, B))                  # emb[it] ^T (pre-relu)
    hprevt_d = inp("hprevt", (H, B))              # h_prev ^T
    hprevown_d = inp("hprevown", (H, BC))         # own batch slice of ^T
    cprev_d = inp("cprev", (B, HC), F32)          # c_prev[:, own H chunk]
    attb_d = inp("attb", (H,), F32)               # att_b
    tanhb_d = inp("tanhb", (H,), F32)             # ctx_b + h2att_b
    alpha_d = inp("alpha", (H,))                  # alpha_W[0]

    outh_d = nc.declare_dram_parameter("out_h", [B, HC], F32, isOutput=True)
    outc_d = nc.declare_dram_parameter("out_c", [B, HC], F32, isOutput=True)

    with tile.TileContext(nc) as tc:
        with (
            tc.tile_pool(name="sb", bufs=1) as sb,
            tc.tile_pool(name="ps", bufs=1, space="PSUM") as ps,
            tc.tile_pool(name="dram", bufs=1, space="DRAM") as dram,
        ):
            # ---- phase B: mm1  att_feats^T = relu(attW @ V^T + b) -----
            # emission order = DMA priority: first weights/Vt for mm1.
            attb_sb = sb.tile([128, AT], F32)
            nc.sync.dma_start(out=attb_sb[:],
                              in_=attb_d.ap().rearrange("(t p) -> p t", p=128))
            wa0 = sb.tile([128, KT, 128], BF, tag="wtile", bufs=3, name="wa0")
            nc.sync.dma_start(out=wa0[:], in_=attw_d.ap()[0])
            # tiny dummy AllGather: pays the cold ncfw/trigger latency early,
            # hidden under mm1, so the real gathers start fast
            warm_in = dram.tile([1, 32], BF)
            warm_out = dram.tile([NCORE, 32], BF, addr_space="Shared")
            warm_sb = sb.tile([1, 32], BF)
            nc.vector.memset(warm_sb[:], 0.0)
            nc.gpsimd.dma_start(out=warm_in[:], in_=warm_sb[:])
            nc.gpsimd.collective_compute(
                "AllGather", ALU.bypass,
                replica_groups=[list(range(NCORE))],
                ins=[warm_in.opt()], outs=[warm_out.opt()])
            vt_sb = sb.tile([128, KT, R], BF)
            vt_v = vt_d.ap().rearrange("(k p) r -> p k r", p=128)
            for k in range(KT):
                eng = nc.sync if k % 2 == 0 else nc.scalar
                eng.dma_start(out=vt_sb[:, k, :], in_=vt_v[:, k, :])
            a2c_sb = sb.tile([128, KT, 2 * HC], BF)
            nc.scalar.dma_start(
                out=a2c_sb[:],
                in_=a2c_d.ap()[0:H, :].rearrange("(k p) n -> p k n", p=128))
            att_f = sb.tile([128, AT, R], BF)  # att_feats^T, bf16

            for m in range(AT):
                if m == 0:
                    wa = wa0
                else:
                    wa = sb.tile([128, KT, 128], BF, tag="wtile", bufs=3,
                                 name=f"wa{m}")
                    nc.sync.dma_start(out=wa[:], in_=attw_d.ap()[m])
                psbs = [ps.tile([128, rsz], F32, tag="acc", bufs=7,
                                name=f"ps_mm1_{m}_{roff}")
                        for (roff, rsz, bo, bt) in RT]
                for k in range(KT):
                    for ni, (roff, rsz, bo, bt) in enumerate(RT):
                        nc.tensor.matmul(psbs[ni][:], wa[:, k],
                                         vt_sb[:, k, roff:roff + rsz],
                                         start=(k == 0), stop=(k == KT - 1))
                for ni, (roff, rsz, bo, bt) in enumerate(RT):
                    nc.scalar.activation(att_f[:, m, roff:roff + rsz], psbs[ni][:],
                                         AF.Relu, bias=attb_sb[:, m:m + 1])

            # ---- phase A: att_h (b-stationary) + PE transpose ---------
            hprevown_sb = sb.tile([128, KT, BC], BF)
            nc.sync.dma_start(out=hprevown_sb[:],
                              in_=hprevown_d.ap().rearrange("(k p) b -> p k b", p=128))
            from concourse import masks
            ident32 = sb.tile([32, 32], BF)
            masks.make_identity(nc, ident32[:])
            h2att_v = h2attw_d.ap().rearrange("(k p) n -> p k n", p=128)
            ah_sbA = sb.tile([32, H], BF)
            for n in range(4):
                psa = ps.tile([32, 512], F32, tag="acc", bufs=7, name=f"ps_ah_{n}")
                for k in range(KT):
                    whn = sb.tile([128, 512], BF, tag="whtile", bufs=4,
                                  name=f"wh{n}_{k}")
                    nc.scalar.dma_start(out=whn[:],
                                         in_=h2att_v[:, k, n * 512:(n + 1) * 512])
                    nc.tensor.matmul(psa[:], hprevown_sb[:, k, :], whn[:],
                                     start=(k == 0), stop=(k == KT - 1))
                nc.vector.tensor_copy(ah_sbA[:, n * 512:(n + 1) * 512], psa[:])
            atthT = sb.tile([128, AT, BC, 1], F32)
            for m in range(AT):
                pst = ps.tile([128, BC], BF, tag="acc", bufs=7, name=f"ps_at_{m}")
                nc.tensor.transpose(pst[:], ah_sbA[:, m * 128:(m + 1) * 128],
                                    ident32[:])
                nc.vector.tensor_copy(atthT[:, m, :, 0], pst[:])

            # ---- phase C: mm2 + tanh + scores -------------------------
            tanhb_sb = sb.tile([128, AT], F32)
            nc.sync.dma_start(out=tanhb_sb[:],
                              in_=tanhb_d.ap().rearrange("(t p) -> p t", p=128))
            alpha_sb = sb.tile([128, AT], BF)
            nc.sync.dma_start(out=alpha_sb[:],
                              in_=alpha_d.ap().rearrange("(t p) -> p t", p=128))
            # all 4 r-tile score accumulators share ONE psum bank at
            # partitions 0/32/64/96 (M=1 col-group matmuls)
            scb = ps.tile([97, 512], F32, tag="sc", bufs=1)
            for m2 in range(AT):
                wc = sb.tile([128, KT, 128], BF, tag="wtile", bufs=3, name=f"wc{m2}")
                nc.sync.dma_start(out=wc[:], in_=ctxw_d.ap()[m2])
                ps2s = [ps.tile([128, rsz], F32, tag="acc", bufs=7,
                                name=f"ps_mm2_{m2}_{roff}")
                        for (roff, rsz, bo, bt) in RT]
                for k in range(KT):
                    for ni, (roff, rsz, bo, bt) in enumerate(RT):
                        nc.tensor.matmul(ps2s[ni][:], wc[:, k],
                                         att_f[:, k, roff:roff + rsz],
                                         start=(k == 0), stop=(k == KT - 1))
                for ni, (roff, rsz, bo, bt) in enumerate(RT):
                    ps2 = ps2s[ni]
                    nc.vector.tensor_tensor(
                        out=ps2.rearrange("p (b s) -> p b s", s=S),
                        in0=ps2.rearrange("p (b s) -> p b s", s=S),
                        in1=atthT[:, m2, bo:bo + bt, :].broadcast_to((128, bt, S)),
                        op=ALU.add)
                    dot_sb = sb.tile([128, rsz], BF, tag="dot", bufs=3,
                                     name=f"dot{m2}_{roff}")
                    nc.scalar.activation(dot_sb[:], ps2[:], AF.Tanh,
                                         bias=tanhb_sb[:, m2:m2 + 1])
                    nc.tensor.matmul(scb[32 * ni:32 * ni + 1, 0:rsz],
                                     alpha_sb[:, m2:m2 + 1], dot_sb[:],
                                     start=(m2 == 0), stop=(m2 == AT - 1),
                                     tile_position=(0, 32 * ni))

            # ---- phase D: softmax over s (per b) ----------------------
            scores_sb = sb.tile([1, R], BF)
            for ni, (roff, rsz, bo, bt) in enumerate(RT):
                nc.scalar.activation(scores_sb[:, roff:roff + rsz],
                                     scb[32 * ni:32 * ni + 1, 0:rsz], AF.Exp)
            ssum = sb.tile([1, BC], F32)
            nc.vector.tensor_reduce(ssum[:],
                                    scores_sb.rearrange("p (b s) -> p b s", s=S),
                                    axis=mybir.AxisListType.X, op=ALU.add)
            rinv = sb.tile([1, BC, 1], F32)
            nc.vector.reciprocal(rinv[:, :, 0], ssum[:])
            w_bf = sb.tile([1, R], BF)
            nc.vector.tensor_tensor(
                out=w_bf.rearrange("p (b s) -> p b s", s=S),
                in0=scores_sb.rearrange("p (b s) -> p b s", s=S),
                in1=rinv.broadcast_to((1, BC, S)),
                op=ALU.mult)
            w_row = dram.tile([1, R], BF)
            nc.gpsimd.dma_start(out=w_row[:], in_=w_bf[:])
            w_exp = sb.tile([128, R], BF)
            nc.gpsimd.dma_start(out=w_exp[:], in_=w_row.broadcast_to((128, R)))

            # ---- phase E: att_res^T + 2 half AllGathers ---------------
            # ar_all[p, m, b]; DRAM halves laid out [p][kk*b] so both the
            # store and the post-gather reload use >=512B contiguous runs.
            ar_half = [sb.tile([128, AT // 2, BC], BF, name=f"ar_half{h}")
                       for h in range(2)]
            for m in range(AT):
                prodm = sb.tile([128, R], BF, tag="prod", bufs=1, name=f"prod{m}")
                nc.vector.tensor_tensor(out=prodm[:], in0=att_f[:, m, :],
                                        in1=w_exp[:], op=ALU.mult)
                arf = sb.tile([128, BC], F32, tag="arf", bufs=2, name=f"arf{m}")
                nc.vector.tensor_reduce(arf[:],
                                        prodm.rearrange("p (b s) -> p b s", s=S),
                                        axis=mybir.AxisListType.X, op=ALU.add)
                nc.scalar.activation(ar_half[m // 8][:, m % 8, :], arf[:], AF.Copy)
            ar_loc = [dram.tile([128, KT // 2 * BC], BF, name=f"ar_loc{h}")
                      for h in range(2)]
            ar_g = [dram.tile([NCORE * 128, KT // 2 * BC], BF,
                              addr_space="Shared", name=f"ar_g{h}")
                    for h in range(2)]
            arT = [sb.tile([128, KT // 2, NCORE, BC], BF, name=f"arT{h}")
                   for h in range(2)]
            for h in range(2):
                nc.gpsimd.dma_start(
                    out=ar_loc[h].rearrange("p (kk b) -> p kk b", b=BC),
                    in_=ar_half[h][:])
                nc.gpsimd.collective_compute(
                    "AllGather", ALU.bypass,
                    replica_groups=[list(range(NCORE))],
                    ins=[ar_loc[h].opt()], outs=[ar_g[h].opt()])
                for c in range(NCORE):
                    eng = nc.scalar if c % 2 == 0 else nc.sync
                    eng.dma_start(
                        out=arT[h][:, :, c, :],
                        in_=ar_g[h][c * 128:(c + 1) * 128, :].rearrange(
                            "p (kk b) -> p kk b", b=BC))

            # ---- phase F: LSTM (tensor-parallel over H chunk) ---------
            xt_sb = sb.tile([128, KT, B], BF)
            nc.sync.dma_start(out=xt_sb[:],
                              in_=embt_d.ap().rearrange("(k p) b -> p k b", p=128))
            nc.scalar.activation(xt_sb[:], xt_sb[:], AF.Relu)
            hprev_sb = sb.tile([128, KT, B], BF)
            nc.sync.dma_start(out=hprev_sb[:],
                              in_=hprevt_d.ap().rearrange("(k p) b -> p k b", p=128))
            cprev_sb = sb.tile([128, 2, HC], F32)
            nc.sync.dma_start(out=cprev_sb[:],
                              in_=cprev_d.ap().rearrange("(m p) h -> p m h", p=128))
            ones_sb = sb.tile([1, B], BF)
            nc.vector.memset(ones_sb[:], 1.0)
            i2hb_sb = sb.tile([1, 5 * HC], BF)
            nc.sync.dma_start(out=i2hb_sb[:], in_=i2h_d.ap()[H:H + 1, :])
            a2cb_sb = sb.tile([1, 2 * HC], BF)
            nc.sync.dma_start(out=a2cb_sb[:], in_=a2c_d.ap()[H:H + 1, :])
            i2h_v = i2h_d.ap()[0:H, :].rearrange("(k p) n -> p k n", p=128)
            h2h_v = h2h_d.ap().rearrange("(k p) n -> p k n", p=128)

            LT = [(0, 512), (512, 512), (1024, 256)]
            psl = {}
            sig_if, sig_o, intr_a = {}, {}, {}
            for (noff, nsz) in LT:
                for mb in range(2):
                    psl[(noff, mb)] = ps.tile([128, nsz], F32, tag="acc", bufs=7,
                                              name=f"ps_l{noff}_{mb}")
                for k in range(KT):
                    ri = sb.tile([128, nsz], BF, tag="lrhs", bufs=4,
                                 name=f"ri{noff}_{k}")
                    nc.sync.dma_start(out=ri[:], in_=i2h_v[:, k, noff:noff + nsz])
                    for mb in range(2):
                        nc.tensor.matmul(psl[(noff, mb)][:],
                                         xt_sb[:, k, mb * 128:(mb + 1) * 128],
                                         ri[:], start=(k == 0), stop=False)
                for k in range(KT):
                    rh = sb.tile([128, nsz], BF, tag="lrhs", bufs=4,
                                 name=f"rh{noff}_{k}")
                    nc.sync.dma_start(out=rh[:], in_=h2h_v[:, k, noff:noff + nsz])
                    for mb in range(2):
                        nc.tensor.matmul(psl[(noff, mb)][:],
                                         hprev_sb[:, k, mb * 128:(mb + 1) * 128],
                                         rh[:], start=False, stop=False)
                for mb in range(2):
                    nc.tensor.matmul(psl[(noff, mb)][:],
                                     ones_sb[:, mb * 128:(mb + 1) * 128],
                                     i2hb_sb[:, noff:noff + nsz],
                                     start=False, stop=(noff == 0))
                if noff >= 512:
                    # a2c contribution lands in in_tr cols [768:1280) of 5*HC
                    if noff == 512:
                        dsl, acs = slice(256, 512), slice(0, 256)
                    else:
                        dsl, acs = slice(0, 256), slice(256, 512)
                    for k in range(KT):
                        h, kk = divmod(k, KT // 2)
                        for mb in range(2):
                            nc.tensor.matmul(psl[(noff, mb)][:, dsl],
                                             arT[h][:, kk, mb * 4:(mb + 1) * 4, :],
                                             a2c_sb[:, k, acs],
                                             start=False, stop=False)
                    for mb in range(2):
                        nc.tensor.matmul(psl[(noff, mb)][:, dsl],
                                         ones_sb[:, mb * 128:(mb + 1) * 128],
                                         a2cb_sb[:, acs], start=False, stop=True)
                # drain
                for mb in range(2):
                    p_ = psl[(noff, mb)]
                    if noff == 0:
                        t_ = sb.tile([128, 512], F32, tag="sif", bufs=2,
                                     name=f"sif{mb}")
                        nc.scalar.activation(t_[:], p_[:], AF.Sigmoid)
                        sig_if[mb] = t_
                    elif noff == 512:
                        t_ = sb.tile([128, 256], F32, tag="sio", bufs=2,
                                     name=f"sio{mb}")
                        nc.scalar.activation(t_[:], p_[:, 0:256], AF.Sigmoid)
                        sig_o[mb] = t_
                        t2_ = sb.tile([128, 256], F32, tag="itra", bufs=2,
                                      name=f"itra{mb}")
                        nc.vector.tensor_copy(t2_[:], p_[:, 256:512])
                        intr_a[mb] = t2_

            for mb in range(2):
                bsl = slice(mb * 128, (mb + 1) * 128)
                in_tr = sb.tile([128, 256], F32, tag="intr", bufs=2, name=f"intr{mb}")
                nc.vector.tensor_tensor(out=in_tr[:],
                                        in0=intr_a[mb][:],
                                        in1=psl[(1024, mb)][:, 0:256], op=ALU.max)
                t_fc = sb.tile([128, 256], F32, tag="tfc", bufs=2, name=f"tfc{mb}")
                nc.vector.tensor_tensor(out=t_fc[:], in0=sig_if[mb][:, 256:512],
                                        in1=cprev_sb[:, mb, :], op=ALU.mult)
                t_ii = sb.tile([128, 256], F32, tag="tii", bufs=2, name=f"tii{mb}")
                nc.vector.tensor_tensor(out=t_ii[:], in0=sig_if[mb][:, 0:256],
                                        in1=in_tr[:], op=ALU.mult)
                next_c = sb.tile([128, 256], F32, tag="nc", bufs=2, name=f"nc{mb}")
                nc.vector.tensor_tensor(out=next_c[:], in0=t_fc[:], in1=t_ii[:],
                                        op=ALU.add)
                nc.gpsimd.dma_start(out=outc_d.ap()[bsl, :], in_=next_c[:])
                tanh_c = sb.tile([128, 256], F32, tag="tfc", bufs=2, name=f"tc{mb}")
                nc.scalar.activation(tanh_c[:], next_c[:], AF.Tanh)
                next_h = sb.tile([128, 256], F32, tag="tii", bufs=2, name=f"nh{mb}")
                nc.vector.tensor_tensor(out=next_h[:], in0=sig_o[mb][:],
                                        in1=tanh_c[:], op=ALU.mult)
                nc.gpsimd.dma_start(out=outh_d.ap()[bsl, :], in_=next_h[:])

    nc.compile()
    return nc


def _tile4(wt):
    """(2048,2048) [f,out] -> (16 m,128 p,16 k,128 j) contiguous bf16."""
    return np.ascontiguousarray(
        wt.reshape(KT, 128, AT, 128).transpose(2, 1, 0, 3)).astype(BF16)


def _bf(a):
    return np.ascontiguousarray(a).astype(BF16)


def prepare_inputs(V, v_g, state, fc_W, fc_b, att_W, att_b, emb, ctx_W, ctx_b,
                   a2c_W, a2c_b, i2h_W, i2h_b, h2h_W, h2h_b,
                   h2att_W, h2att_b, alpha_W, alpha_b, captions, time):
    t = int(np.asarray(time))
    V = np.asarray(V, np.float32)
    state = np.asarray(state, np.float32)
    if t == 0:
        state = np.zeros_like(state)
    h_prev = state[0, -1]
    c_prev = state[1, -1]
    it = np.asarray(captions, np.int64)[:, t]
    emb_it = np.asarray(emb, np.float32)[it]          # (B, H)

    attw4 = _tile4(np.asarray(att_W, np.float32).T)
    ctxw4 = _tile4(np.asarray(ctx_W, np.float32).T)
    h2attw4 = _bf(np.asarray(h2att_W, np.float32).T)
    embt = _bf(emb_it.T)
    hprevt = _bf(h_prev.T)
    attb = np.ascontiguousarray(att_b, np.float32)
    tanhb = np.ascontiguousarray(np.asarray(ctx_b, np.float32)
                                 + np.asarray(h2att_b, np.float32))
    alpha = _bf(np.asarray(alpha_W, np.float32)[0])
    ib = np.asarray(i2h_b, np.float32) + np.asarray(h2h_b, np.float32)
    i2h_W = np.asarray(i2h_W, np.float32)
    h2h_W = np.asarray(h2h_W, np.float32)
    a2c_W = np.asarray(a2c_W, np.float32)
    a2c_b = np.asarray(a2c_b, np.float32)

    in_maps = []
    for c in range(NCORE):
        bs = slice(c * BC, (c + 1) * BC)
        hs = [slice(g * H + c * HC, g * H + (c + 1) * HC) for g in range(5)]
        i2h_blk = np.concatenate([i2h_W[s] for s in hs], 0)       # (1280, H)
        h2h_blk = np.concatenate([h2h_W[s] for s in hs], 0)
        ib_blk = np.concatenate([ib[s] for s in hs], 0)           # (1280,)
        a2c_blk = np.concatenate([a2c_W[s] for s in hs[:2]], 0)   # (512, H)
        a2cb_blk = np.concatenate([a2c_b[s] for s in hs[:2]], 0)
        vt = _bf(V[bs].transpose(2, 0, 1).reshape(H, R))
        in_maps.append({
            "vt": vt,
            "attw": attw4, "ctxw": ctxw4, "h2attw": h2attw4,
            "i2h": _bf(np.vstack([i2h_blk.T, ib_blk[None, :]])),
            "h2h": _bf(h2h_blk.T),
            "a2c": _bf(np.vstack([a2c_blk.T, a2cb_blk[None, :]])),
            "embt": embt, "hprevt": hprevt,
            "hprevown": np.ascontiguousarray(hprevt[:, bs]),
            "cprev": np.ascontiguousarray(c_prev[:, c * HC:(c + 1) * HC],
                                          ).astype(np.float32),
            "attb": attb, "tanhb": tanhb, "alpha": alpha,
        })
    return in_maps


def kernel(**inputs):
    in_maps = prepare_inputs(**inputs)
    if "nc" not in _NC_CACHE:
        _NC_CACHE["nc"] = build_nc()
    nc = _NC_CACHE["nc"]
    res = run_bass_kernel_spmd(nc, in_maps, core_ids=list(range(NCORE)))
    next_h = np.concatenate([res.results[c]["out_h"] for c in range(NCORE)], 1)
    next_c = np.concatenate([res.results[c]["out_c"] for c in range(NCORE)], 1)
    next_h = np.asarray(next_h, np.float32)
    next_c = np.asarray(next_c, np.float32)
    return next_h[:, None, :], next_h[None], next_c[None]


# revision 26
# speedup vs baseline: 1.0524x; 1.0425x over previous
"""Att2in2Core fused kernel for 8 Trainium2 NeuronCores.

Sharding: attention part data-parallel over batch (32 rows/core); the
LSTM-with-maxout part tensor-parallel over H (256 cols/core spanning all
five i2h/h2h gate groups), bridged by two bf16 AllGathers of att_res.

Self-contained: hardcodes B=256, S=49, H=2048 and the sharding. The
only runtime-variant input handling is `time` (captions column select +
state zeroing at t==0), done host-side as input preparation.
"""
import os
import sys

os.environ.setdefault("NEURON_RT_DBG_RDH_CC", "0")
sys.path.insert(0, "/opt/trn_rl_repo")

import numpy as np
import ml_dtypes

import concourse.bass as bass
import concourse.bacc as bacc
import concourse.mybir as mybir
import concourse.tile as tile
from concourse.bass_utils import run_bass_kernel_spmd

BF16 = ml_dtypes.bfloat16
B, S, H = 256, 49, 2048
NCORE = 8
BC = B // NCORE            # 32 batch rows per core
HC = H // NCORE            # 256 H cols per core (LSTM shard)
R = S * BC                 # 1568 attention rows per core (b-major)
KT = H // 128              # 16 contraction tiles
AT = H // 128              # 16 output a-tiles
# r-tiles (b-major, r = b*49 + s): (roff, rsz, b_off, b_cnt)
RT = [(0, 490, 0, 10), (490, 490, 10, 10), (980, 490, 20, 10), (1470, 98, 30, 2)]
F32 = mybir.dt.float32
BF = mybir.dt.bfloat16
AF = mybir.ActivationFunctionType
ALU = mybir.AluOpType

_NC_CACHE = {}


def build_nc():
    nc = bacc.Bacc("TRN2", target_bir_lowering=False, debug=False,
                   num_devices=NCORE)

    def inp(name, shape, dt=BF):
        return nc.declare_dram_parameter(name, list(shape), dt, isOutput=False)

    vt_d = inp("vt", (H, R))                      # V shard ^T, b-major cols
    attw_d = inp("attw", (AT, 128, KT, 128))      # pre-tiled [m][p][k][j]
    ctxw_d = inp("ctxw", (AT, 128, KT, 128))
    h2attw_d = inp("h2attw", (AT, 128, KT, 128))
    i2h_d = inp("i2h", (H + 1, 5 * HC))           # shard ^T + bias row
    h2h_d = inp("h2h", (H, 5 * HC))               # shard ^T
    a2c_d = inp("a2c", (H + 1, 2 * HC))           # shard ^T + bias row
    embt_d = inp("embt", (H, B))                  # emb[it] ^T (pre-relu)
    hprevt_d = inp("hprevt", (H, B))              # h_prev ^T
    hprevown_d = inp("hprevown", (H, BC))         # own batch slice of ^T
    cprev_d = inp("cprev", (B, HC), F32)          # c_prev[:, own H chunk]
    attb_d = inp("attb", (H,), F32)               # att_b
    tanhb_d = inp("tanhb", (H,), F32)             # ctx_b + h2att_b
    alpha_d = inp("alpha", (H,))                  # alpha_W[0]

    outh_d = nc.declare_dram_parameter("out_h", [B, HC], F32, isOutput=True)
    outc_d = nc.declare_dram_parameter("out_c", [B, HC], F32, isOutput=True)

    with tile.TileContext(nc) as tc:
        with (
            tc.tile_pool(name="sb", bufs=1) as sb,
            tc.tile_pool(name="ps", bufs=1, space="PSUM") as ps,
            tc.tile_pool(name="dram", bufs=1, space="DRAM") as dram,
        ):
            # ---- phase B: mm1  att_feats^T = relu(attW @ V^T + b) -----
            # emission order = DMA priority: first weights/Vt for mm1.
            attb_sb = sb.tile([128, AT], F32)
            nc.sync.dma_start(out=attb_sb[:],
                              in_=attb_d.ap().rearrange("(t p) -> p t", p=128))
            wa0 = sb.tile([128, KT, 128], BF, tag="wtile", bufs=3, name="wa0")
            nc.sync.dma_start(out=wa0[:], in_=attw_d.ap()[0])
            vt_sb = sb.tile([128, KT, R], BF)
            vt_v = vt_d.ap().rearrange("(k p) r -> p k r", p=128)
            for k in range(KT):
                eng = nc.sync if k % 2 == 0 else nc.scalar
                eng.dma_start(out=vt_sb[:, k, :], in_=vt_v[:, k, :])
            # early loads + ncfw warm-up AllGather (hidden under mm1)
            hprev_sb = sb.tile([128, KT, B], BF)
            nc.scalar.dma_start(out=hprev_sb[:],
                                in_=hprevt_d.ap().rearrange("(k p) b -> p k b", p=128))
            warm_in = dram.tile([1, 32], BF)
            warm_out = dram.tile([NCORE, 32], BF, addr_space="Shared")
            warm_sb = sb.tile([1, 32], BF)
            nc.vector.memset(warm_sb[:], 0.0)
            nc.gpsimd.dma_start(out=warm_in[:], in_=warm_sb[:])
            nc.gpsimd.collective_compute(
                "AllGather", ALU.bypass,
                replica_groups=[list(range(NCORE))],
                ins=[warm_in.opt()], outs=[warm_out.opt()])
            a2c_sb = sb.tile([128, KT, 2 * HC], BF)
            nc.scalar.dma_start(
                out=a2c_sb[:],
                in_=a2c_d.ap()[0:H, :].rearrange("(k p) n -> p k n", p=128))
            att_f = sb.tile([128, AT, R], BF)  # att_feats^T, bf16

            for m in range(AT):
                if m == 0:
                    wa = wa0
                else:
                    wa = sb.tile([128, KT, 128], BF, tag="wtile", bufs=3,
                                 name=f"wa{m}")
                    nc.sync.dma_start(out=wa[:], in_=attw_d.ap()[m])
                psbs = [ps.tile([128, rsz], F32, tag="acc", bufs=7,
                                name=f"ps_mm1_{m}_{roff}")
                        for (roff, rsz, bo, bt) in RT]
                for k in range(KT):
                    for ni, (roff, rsz, bo, bt) in enumerate(RT):
                        nc.tensor.matmul(psbs[ni][:], wa[:, k],
                                         vt_sb[:, k, roff:roff + rsz],
                                         start=(k == 0), stop=(k == KT - 1))
                for ni, (roff, rsz, bo, bt) in enumerate(RT):
                    nc.scalar.activation(att_f[:, m, roff:roff + rsz], psbs[ni][:],
                                         AF.Relu, bias=attb_sb[:, m:m + 1])

            # ---- phase A: att_h^T (own batch cols, DP) ----------------
            hprevown_sb = sb.tile([128, KT, BC], BF)
            nc.sync.dma_start(out=hprevown_sb[:],
                              in_=hprevown_d.ap().rearrange("(k p) b -> p k b", p=128))
            atthT = sb.tile([128, AT, BC, 1], F32)
            for m in range(AT):
                wh = sb.tile([128, KT, 128], BF, tag="wtile", bufs=3, name=f"wh{m}")
                nc.sync.dma_start(out=wh[:], in_=h2attw_d.ap()[m])
                psa = ps.tile([128, BC], F32, tag="acc", bufs=7, name=f"ps_ah_{m}")
                for k in range(KT):
                    nc.tensor.matmul(psa[:], wh[:, k], hprevown_sb[:, k],
                                     start=(k == 0), stop=(k == KT - 1))
                nc.vector.tensor_copy(atthT[:, m, :, 0], psa[:])

            # ---- phase C: mm2 + tanh + scores -------------------------
            tanhb_sb = sb.tile([128, AT], F32)
            nc.sync.dma_start(out=tanhb_sb[:],
                              in_=tanhb_d.ap().rearrange("(t p) -> p t", p=128))
            alpha_sb = sb.tile([128, AT], BF)
            nc.sync.dma_start(out=alpha_sb[:],
                              in_=alpha_d.ap().rearrange("(t p) -> p t", p=128))
            # all 4 r-tile score accumulators share ONE psum bank at
            # partitions 0/32/64/96 (M=1 col-group matmuls)
            scb = ps.tile([97, 512], F32, tag="sc", bufs=1)
            for m2 in range(AT):
                wc = sb.tile([128, KT, 128], BF, tag="wtile", bufs=3, name=f"wc{m2}")
                nc.sync.dma_start(out=wc[:], in_=ctxw_d.ap()[m2])
                ps2s = [ps.tile([128, rsz], F32, tag="acc", bufs=7,
                                name=f"ps_mm2_{m2}_{roff}")
                        for (roff, rsz, bo, bt) in RT]
                for k in range(KT):
                    for ni, (roff, rsz, bo, bt) in enumerate(RT):
                        nc.tensor.matmul(ps2s[ni][:], wc[:, k],
                                         att_f[:, k, roff:roff + rsz],
                                         start=(k == 0), stop=(k == KT - 1))
                for ni, (roff, rsz, bo, bt) in enumerate(RT):
                    ps2 = ps2s[ni]
                    nc.vector.tensor_tensor(
                        out=ps2.rearrange("p (b s) -> p b s", s=S),
                        in0=ps2.rearrange("p (b s) -> p b s", s=S),
                        in1=atthT[:, m2, bo:bo + bt, :].broadcast_to((128, bt, S)),
                        op=ALU.add)
                    dot_sb = sb.tile([128, rsz], BF, tag="dot", bufs=3,
                                     name=f"dot{m2}_{roff}")
                    nc.scalar.activation(dot_sb[:], ps2[:], AF.Tanh,
                                         bias=tanhb_sb[:, m2:m2 + 1])
                    nc.tensor.matmul(scb[32 * ni:32 * ni + 1, 0:rsz],
                                     alpha_sb[:, m2:m2 + 1], dot_sb[:],
                                     start=(m2 == 0), stop=(m2 == AT - 1),
                                     tile_position=(0, 32 * ni))

            # ---- phase D: softmax over s (per b) ----------------------
            scores_sb = sb.tile([1, R], BF)
            for ni, (roff, rsz, bo, bt) in enumerate(RT):
                nc.scalar.activation(scores_sb[:, roff:roff + rsz],
                                     scb[32 * ni:32 * ni + 1, 0:rsz], AF.Exp)
            ssum = sb.tile([1, BC], F32)
            nc.vector.tensor_reduce(ssum[:],
                                    scores_sb.rearrange("p (b s) -> p b s", s=S),
                                    axis=mybir.AxisListType.X, op=ALU.add)
            rinv = sb.tile([1, BC, 1], F32)
            nc.vector.reciprocal(rinv[:, :, 0], ssum[:])
            w_bf = sb.tile([1, R], BF)
            nc.vector.tensor_tensor(
                out=w_bf.rearrange("p (b s) -> p b s", s=S),
                in0=scores_sb.rearrange("p (b s) -> p b s", s=S),
                in1=rinv.broadcast_to((1, BC, S)),
                op=ALU.mult)
            w_row = dram.tile([1, R], BF)
            nc.gpsimd.dma_start(out=w_row[:], in_=w_bf[:])
            w_exp = sb.tile([128, R], BF)
            nc.gpsimd.dma_start(out=w_exp[:], in_=w_row.broadcast_to((128, R)))

            # ---- phase E: att_res^T + 2 half AllGathers ---------------
            # ar_all[p, m, b]; DRAM halves laid out [p][kk*b] so both the
            # store and the post-gather reload use >=512B contiguous runs.
            ar_half = [sb.tile([128, AT // 2, BC], BF, name=f"ar_half{h}")
                       for h in range(2)]
            for m in range(AT):
                prodm = sb.tile([128, R], BF, tag="prod", bufs=1, name=f"prod{m}")
                nc.vector.tensor_tensor(out=prodm[:], in0=att_f[:, m, :],
                                        in1=w_exp[:], op=ALU.mult)
                arf = sb.tile([128, BC], F32, tag="arf", bufs=2, name=f"arf{m}")
                nc.vector.tensor_reduce(arf[:],
                                        prodm.rearrange("p (b s) -> p b s", s=S),
                                        axis=mybir.AxisListType.X, op=ALU.add)
                nc.scalar.activation(ar_half[m // 8][:, m % 8, :], arf[:], AF.Copy)
            ar_loc = [dram.tile([128, KT // 2 * BC], BF, name=f"ar_loc{h}")
                      for h in range(2)]
            ar_g = [dram.tile([NCORE * 128, KT // 2 * BC], BF,
                              addr_space="Shared", name=f"ar_g{h}")
                    for h in range(2)]
            arT = [sb.tile([128, KT // 2, NCORE, BC], BF, name=f"arT{h}")
                   for h in range(2)]
            for h in range(2):
                nc.gpsimd.dma_start(
                    out=ar_loc[h].rearrange("p (kk b) -> p kk b", b=BC),
                    in_=ar_half[h][:])
                nc.gpsimd.collective_compute(
                    "AllGather", ALU.bypass,
                    replica_groups=[list(range(NCORE))],
                    ins=[ar_loc[h].opt()], outs=[ar_g[h].opt()])
                for c in range(NCORE):
                    eng = nc.scalar if c % 2 == 0 else nc.sync
                    eng.dma_start(
                        out=arT[h][:, :, c, :],
                        in_=ar_g[h][c * 128:(c + 1) * 128, :].rearrange(
                            "p (kk b) -> p kk b", b=BC))

            # ---- phase F: LSTM (tensor-parallel over H chunk) ---------
            xt_sb = sb.tile([128, KT, B], BF)
            nc.sync.dma_start(out=xt_sb[:],
                              in_=embt_d.ap().rearrange("(k p) b -> p k b", p=128))
            nc.scalar.activation(xt_sb[:], xt_sb[:], AF.Relu)
            hprev_sb = sb.tile([128, KT, B], BF)
            nc.sync.dma_start(out=hprev_sb[:],
                              in_=hprevt_d.ap().rearrange("(k p) b -> p k b", p=128))
            cprev_sb = sb.tile([128, 2, HC], F32)
            nc.sync.dma_start(out=cprev_sb[:],
                              in_=cprev_d.ap().rearrange("(m p) h -> p m h", p=128))
            ones_sb = sb.tile([1, B], BF)
            nc.vector.memset(ones_sb[:], 1.0)
            i2hb_sb = sb.tile([1, 5 * HC], BF)
            nc.sync.dma_start(out=i2hb_sb[:], in_=i2h_d.ap()[H:H + 1, :])
            a2cb_sb = sb.tile([1, 2 * HC], BF)
            nc.sync.dma_start(out=a2cb_sb[:], in_=a2c_d.ap()[H:H + 1, :])
            i2h_v = i2h_d.ap()[0:H, :].rearrange("(k p) n -> p k n", p=128)
            h2h_v = h2h_d.ap().rearrange("(k p) n -> p k n", p=128)

            LT = [(0, 512), (512, 512), (1024, 256)]
            psl = {}
            sig_if, sig_o, intr_a = {}, {}, {}
            for (noff, nsz) in LT:
                for mb in range(2):
                    psl[(noff, mb)] = ps.tile([128, nsz], F32, tag="acc", bufs=7,
                                              name=f"ps_l{noff}_{mb}")
                for k in range(KT):
                    ri = sb.tile([128, nsz], BF, tag="lrhs", bufs=4,
                                 name=f"ri{noff}_{k}")
                    nc.sync.dma_start(out=ri[:], in_=i2h_v[:, k, noff:noff + nsz])
                    for mb in range(2):
                        nc.tensor.matmul(psl[(noff, mb)][:],
                                         xt_sb[:, k, mb * 128:(mb + 1) * 128],
                                         ri[:], start=(k == 0), stop=False)
                for k in range(KT):
                    rh = sb.tile([128, nsz], BF, tag="lrhs", bufs=4,
                                 name=f"rh{noff}_{k}")
                    nc.sync.dma_start(out=rh[:], in_=h2h_v[:, k, noff:noff + nsz])
                    for mb in range(2):
                        nc.tensor.matmul(psl[(noff, mb)][:],
                                         hprev_sb[:, k, mb * 128:(mb + 1) * 128],
                                         rh[:], start=False, stop=False)
                for mb in range(2):
                    nc.tensor.matmul(psl[(noff, mb)][:],
                                     ones_sb[:, mb * 128:(mb + 1) * 128],
                                     i2hb_sb[:, noff:noff + nsz],
                                     start=False, stop=(noff == 0))
                if noff >= 512:
                    # a2c contribution lands in in_tr cols [768:1280) of 5*HC
                    if noff == 512:
                        dsl, acs = slice(256, 512), slice(0, 256)
                    else:
                        dsl, acs = slice(0, 256), slice(256, 512)
                    for k in range(KT):
                        h, kk = divmod(k, KT // 2)
                        for mb in range(2):
                            nc.tensor.matmul(psl[(noff, mb)][:, dsl],
                                             arT[h][:, kk, mb * 4:(mb + 1) * 4, :],
                                             a2c_sb[:, k, acs],
                                             start=False, stop=False)
                    for mb in range(2):
                        nc.tensor.matmul(psl[(noff, mb)][:, dsl],
                                         ones_sb[:, mb * 128:(mb + 1) * 128],
                                         a2cb_sb[:, acs], start=False, stop=True)
                # drain
                for mb in range(2):
                    p_ = psl[(noff, mb)]
                    if noff == 0:
                        t_ = sb.tile([128, 512], F32, tag="sif", bufs=2,
                                     name=f"sif{mb}")
                        nc.scalar.activation(t_[:], p_[:], AF.Sigmoid)
                        sig_if[mb] = t_
                    elif noff == 512:
                        t_ = sb.tile([128, 256], F32, tag="sio", bufs=2,
                                     name=f"sio{mb}")
                        nc.scalar.activation(t_[:], p_[:, 0:256], AF.Sigmoid)
                        sig_o[mb] = t_
                        t2_ = sb.tile([128, 256], F32, tag="itra", bufs=2,
                                      name=f"itra{mb}")
                        nc.vector.tensor_copy(t2_[:], p_[:, 256:512])
                        intr_a[mb] = t2_

            for mb in range(2):
                bsl = slice(mb * 128, (mb + 1) * 128)
                in_tr = sb.tile([128, 256], F32, tag="intr", bufs=2, name=f"intr{mb}")
                nc.vector.tensor_tensor(out=in_tr[:],
                                        in0=intr_a[mb][:],
                                        in1=psl[(1024, mb)][:, 0:256], op=ALU.max)
                t_fc = sb.tile([128, 256], F32, tag="tfc", bufs=2, name=f"tfc{mb}")
                nc.vector.tensor_tensor(out=t_fc[:], in0=sig_if[mb][:, 256:512],
                                        in1=cprev_sb[:, mb, :], op=ALU.mult)
                t_ii = sb.tile([128, 256], F32, tag="tii", bufs=2, name=f"tii{mb}")
                nc.vector.tensor_tensor(out=t_ii[:], in0=sig_if[mb][:, 0:256],
                                        in1=in_tr[:], op=ALU.mult)
                next_c = sb.tile([128, 256], F32, tag="nc", bufs=2, name=f"nc{mb}")
                nc.vector.tensor_tensor(out=next_c[:], in0=t_fc[:], in1=t_ii[:],
                                        op=ALU.add)
                nc.gpsimd.dma_start(out=outc_d.ap()[bsl, :], in_=next_c[:])
                tanh_c = sb.tile([128, 256], F32, tag="tfc", bufs=2, name=f"tc{mb}")
                nc.scalar.activation(tanh_c[:], next_c[:], AF.Tanh)
                next_h = sb.tile([128, 256], F32, tag="tii", bufs=2, name=f"nh{mb}")
                nc.vector.tensor_tensor(out=next_h[:], in0=sig_o[mb][:],
                                        in1=tanh_c[:], op=ALU.mult)
                nc.gpsimd.dma_start(out=outh_d.ap()[bsl, :], in_=next_h[:])

    nc.compile()
    return nc


def _tile4(wt):
    """(2048,2048) [f,out] -> (16 m,128 p,16 k,128 j) contiguous bf16."""
    return np.ascontiguousarray(
        wt.reshape(KT, 128, AT, 128).transpose(2, 1, 0, 3)).astype(BF16)


def _bf(a):
    return np.ascontiguousarray(a).astype(BF16)


def prepare_inputs(V, v_g, state, fc_W, fc_b, att_W, att_b, emb, ctx_W, ctx_b,
                   a2c_W, a2c_b, i2h_W, i2h_b, h2h_W, h2h_b,
                   h2att_W, h2att_b, alpha_W, alpha_b, captions, time):
    t = int(np.asarray(time))
    V = np.asarray(V, np.float32)
    state = np.asarray(state, np.float32)
    if t == 0:
        state = np.zeros_like(state)
    h_prev = state[0, -1]
    c_prev = state[1, -1]
    it = np.asarray(captions, np.int64)[:, t]
    emb_it = np.asarray(emb, np.float32)[it]          # (B, H)

    attw4 = _tile4(np.asarray(att_W, np.float32).T)
    ctxw4 = _tile4(np.asarray(ctx_W, np.float32).T)
    h2attw4 = _tile4(np.asarray(h2att_W, np.float32).T)
    embt = _bf(emb_it.T)
    hprevt = _bf(h_prev.T)
    attb = np.ascontiguousarray(att_b, np.float32)
    tanhb = np.ascontiguousarray(np.asarray(ctx_b, np.float32)
                                 + np.asarray(h2att_b, np.float32))
    alpha = _bf(np.asarray(alpha_W, np.float32)[0])
    ib = np.asarray(i2h_b, np.float32) + np.asarray(h2h_b, np.float32)
    i2h_W = np.asarray(i2h_W, np.float32)
    h2h_W = np.asarray(h2h_W, np.float32)
    a2c_W = np.asarray(a2c_W, np.float32)
    a2c_b = np.asarray(a2c_b, np.float32)

    in_maps = []
    for c in range(NCORE):
        bs = slice(c * BC, (c + 1) * BC)
        hs = [slice(g * H + c * HC, g * H + (c + 1) * HC) for g in range(5)]
        i2h_blk = np.concatenate([i2h_W[s] for s in hs], 0)       # (1280, H)
        h2h_blk = np.concatenate([h2h_W[s] for s in hs], 0)
        ib_blk = np.concatenate([ib[s] for s in hs], 0)           # (1280,)
        a2c_blk = np.concatenate([a2c_W[s] for s in hs[:2]], 0)   # (512, H)
        a2cb_blk = np.concatenate([a2c_b[s] for s in hs[:2]], 0)
        vt = _bf(V[bs].transpose(2, 0, 1).reshape(H, R))
        in_maps.append({
            "vt": vt,
            "attw": attw4, "ctxw": ctxw4, "h2attw": h2attw4,
            "i2h": _bf(np.vstack([i2h_blk.T, ib_blk[None, :]])),
            "h2h": _bf(h2h_blk.T),
            "a2c": _bf(np.vstack([a2c_blk.T, a2cb_blk[None, :]])),
            "embt": embt, "hprevt": hprevt,
            "hprevown": np.ascontiguousarray(hprevt[:, bs]),
            "cprev": np.ascontiguousarray(c_prev[:, c * HC:(c + 1) * HC],
                                          ).astype(np.float32),
            "attb": attb, "tanhb": tanhb, "alpha": alpha,
        })
    return in_maps


def kernel(**inputs):
    in_maps = prepare_inputs(**inputs)
    if "nc" not in _NC_CACHE:
        _NC_CACHE["nc"] = build_nc()
    nc = _NC_CACHE["nc"]
    res = run_bass_kernel_spmd(nc, in_maps, core_ids=list(range(NCORE)))
    next_h = np.concatenate([res.results[c]["out_h"] for c in range(NCORE)], 1)
    next_c = np.concatenate([res.results[c]["out_c"] for c in range(NCORE)], 1)
    next_h = np.asarray(next_h, np.float32)
    next_c = np.asarray(next_c, np.float32)
    return next_h[:, None, :], next_h[None], next_c[None]


# revision 27
# speedup vs baseline: 1.1166x; 1.0610x over previous
"""Att2in2Core fused kernel for 8 Trainium2 NeuronCores.

Sharding: attention part data-parallel over batch (32 rows/core); the
LSTM-with-maxout part tensor-parallel over H (256 cols/core spanning all
five i2h/h2h gate groups), bridged by two bf16 AllGathers of att_res.

Self-contained: hardcodes B=256, S=49, H=2048 and the sharding. The
only runtime-variant input handling is `time` (captions column select +
state zeroing at t==0), done host-side as input preparation.
"""
import os
import sys

os.environ["NEURON_RT_DBG_RDH_CC"] = "0"  # mesh beats RDH for our 0.5MB gathers
sys.path.insert(0, "/opt/trn_rl_repo")

import numpy as np
import ml_dtypes

import concourse.bass as bass
import concourse.bacc as bacc
import concourse.mybir as mybir
import concourse.tile as tile
from concourse.bass_utils import run_bass_kernel_spmd

BF16 = ml_dtypes.bfloat16
B, S, H = 256, 49, 2048
NCORE = 8
BC = B // NCORE            # 32 batch rows per core
HC = H // NCORE            # 256 H cols per core (LSTM shard)
R = S * BC                 # 1568 attention rows per core (b-major)
KT = H // 128              # 16 contraction tiles
AT = H // 128              # 16 output a-tiles
# r-tiles (b-major, r = b*49 + s): (roff, rsz, b_off, b_cnt)
RT = [(0, 490, 0, 10), (490, 490, 10, 10), (980, 490, 20, 10), (1470, 98, 30, 2)]
F32 = mybir.dt.float32
BF = mybir.dt.bfloat16
AF = mybir.ActivationFunctionType
ALU = mybir.AluOpType

_NC_CACHE = {}


def build_nc():
    nc = bacc.Bacc("TRN2", target_bir_lowering=False, debug=False,
                   num_devices=NCORE)

    def inp(name, shape, dt=BF):
        return nc.declare_dram_parameter(name, list(shape), dt, isOutput=False)

    vt_d = inp("vt", (H, R))                      # V shard ^T, b-major cols
    attw_d = inp("attw", (AT, 128, KT, 128))      # pre-tiled [m][p][k][j]
    ctxw_d = inp("ctxw", (AT, 128, KT, 128))
    h2attw_d = inp("h2attw", (AT, 128, KT, 128))
    i2h_d = inp("i2h", (H + 1, 5 * HC))           # shard ^T + bias row
    h2h_d = inp("h2h", (H, 5 * HC))               # shard ^T
    a2c_d = inp("a2c", (H + 1, 2 * HC))           # shard ^T + bias row
    embt_d = inp("embt", (H, B))                  # emb[it] ^T (pre-relu)
    hprevt_d = inp("hprevt", (H, B))              # h_prev ^T
    hprevown_d = inp("hprevown", (H, BC))         # own batch slice of ^T
    cprev_d = inp("cprev", (B, HC), F32)          # c_prev[:, own H chunk]
    attb_d = inp("attb", (H,), F32)               # att_b
    tanhb_d = inp("tanhb", (H,), F32)             # ctx_b + h2att_b
    alpha_d = inp("alpha", (H,))                  # alpha_W[0]

    outh_d = nc.declare_dram_parameter("out_h", [B, HC], F32, isOutput=True)
    outc_d = nc.declare_dram_parameter("out_c", [B, HC], F32, isOutput=True)

    with tile.TileContext(nc) as tc:
        with (
            tc.tile_pool(name="sb", bufs=1) as sb,
            tc.tile_pool(name="ps", bufs=1, space="PSUM") as ps,
            tc.tile_pool(name="dram", bufs=1, space="DRAM") as dram,
        ):
            # ---- phase B: mm1  att_feats^T = relu(attW @ V^T + b) -----
            # emission order = DMA priority: first weights/Vt for mm1.
            attb_sb = sb.tile([128, AT], F32)
            nc.sync.dma_start(out=attb_sb[:],
                              in_=attb_d.ap().rearrange("(t p) -> p t", p=128))
            wa0 = sb.tile([128, KT, 128], BF, tag="wtile", bufs=3, name="wa0")
            nc.sync.dma_start(out=wa0[:], in_=attw_d.ap()[0])
            vt_sb = sb.tile([128, KT, R], BF)
            vt_v = vt_d.ap().rearrange("(k p) r -> p k r", p=128)
            for k in range(KT):
                eng = nc.sync if k % 2 == 0 else nc.scalar
                eng.dma_start(out=vt_sb[:, k, :], in_=vt_v[:, k, :])
            # early loads + ncfw warm-up AllGather (hidden under mm1)
            hprev_sb = sb.tile([128, KT, B], BF)
            nc.scalar.dma_start(out=hprev_sb[:],
                                in_=hprevt_d.ap().rearrange("(k p) b -> p k b", p=128))
            warm_in = dram.tile([1, 32], BF)
            warm_out = dram.tile([NCORE, 32], BF, addr_space="Shared")
            warm_sb = sb.tile([1, 32], BF)
            nc.vector.memset(warm_sb[:], 0.0)
            nc.gpsimd.dma_start(out=warm_in[:], in_=warm_sb[:])
            nc.gpsimd.collective_compute(
                "AllGather", ALU.bypass,
                replica_groups=[list(range(NCORE))],
                ins=[warm_in.opt()], outs=[warm_out.opt()])
            a2c_sb = sb.tile([128, KT, 2 * HC], BF)
            nc.scalar.dma_start(
                out=a2c_sb[:],
                in_=a2c_d.ap()[0:H, :].rearrange("(k p) n -> p k n", p=128))
            att_f = sb.tile([128, AT, R], BF)  # att_feats^T, bf16

            for m in range(AT):
                if m == 0:
                    wa = wa0
                else:
                    wa = sb.tile([128, KT, 128], BF, tag="wtile", bufs=3,
                                 name=f"wa{m}")
                    nc.sync.dma_start(out=wa[:], in_=attw_d.ap()[m])
                psbs = [ps.tile([128, rsz], F32, tag="acc", bufs=7,
                                name=f"ps_mm1_{m}_{roff}")
                        for (roff, rsz, bo, bt) in RT]
                for k in range(KT):
                    for ni, (roff, rsz, bo, bt) in enumerate(RT):
                        nc.tensor.matmul(psbs[ni][:], wa[:, k],
                                         vt_sb[:, k, roff:roff + rsz],
                                         start=(k == 0), stop=(k == KT - 1))
                for ni, (roff, rsz, bo, bt) in enumerate(RT):
                    nc.scalar.activation(att_f[:, m, roff:roff + rsz], psbs[ni][:],
                                         AF.Relu, bias=attb_sb[:, m:m + 1])

            # ---- phase A: att_h^T (own batch cols, DP) ----------------
            hprevown_sb = sb.tile([128, KT, BC], BF)
            nc.sync.dma_start(out=hprevown_sb[:],
                              in_=hprevown_d.ap().rearrange("(k p) b -> p k b", p=128))
            atthT = sb.tile([128, AT, BC, 1], F32)
            for m in range(AT):
                wh = sb.tile([128, KT, 128], BF, tag="wtile", bufs=3, name=f"wh{m}")
                nc.sync.dma_start(out=wh[:], in_=h2attw_d.ap()[m])
                psa = ps.tile([128, BC], F32, tag="acc", bufs=7, name=f"ps_ah_{m}")
                for k in range(KT):
                    nc.tensor.matmul(psa[:], wh[:, k], hprevown_sb[:, k],
                                     start=(k == 0), stop=(k == KT - 1))
                nc.vector.tensor_copy(atthT[:, m, :, 0], psa[:])

            # ---- phase C: mm2 + tanh + scores -------------------------
            tanhb_sb = sb.tile([128, AT], F32)
            nc.sync.dma_start(out=tanhb_sb[:],
                              in_=tanhb_d.ap().rearrange("(t p) -> p t", p=128))
            alpha_sb = sb.tile([128, AT], BF)
            nc.sync.dma_start(out=alpha_sb[:],
                              in_=alpha_d.ap().rearrange("(t p) -> p t", p=128))
            # all 4 r-tile score accumulators share ONE psum bank at
            # partitions 0/32/64/96 (M=1 col-group matmuls)
            scb = ps.tile([97, 512], F32, tag="sc", bufs=1)
            for m2 in range(AT):
                wc = sb.tile([128, KT, 128], BF, tag="wtile", bufs=3, name=f"wc{m2}")
                nc.sync.dma_start(out=wc[:], in_=ctxw_d.ap()[m2])
                ps2s = [ps.tile([128, rsz], F32, tag="acc", bufs=7,
                                name=f"ps_mm2_{m2}_{roff}")
                        for (roff, rsz, bo, bt) in RT]
                for k in range(KT):
                    for ni, (roff, rsz, bo, bt) in enumerate(RT):
                        nc.tensor.matmul(ps2s[ni][:], wc[:, k],
                                         att_f[:, k, roff:roff + rsz],
                                         start=(k == 0), stop=(k == KT - 1))
                for ni, (roff, rsz, bo, bt) in enumerate(RT):
                    ps2 = ps2s[ni]
                    nc.vector.tensor_tensor(
                        out=ps2.rearrange("p (b s) -> p b s", s=S),
                        in0=ps2.rearrange("p (b s) -> p b s", s=S),
                        in1=atthT[:, m2, bo:bo + bt, :].broadcast_to((128, bt, S)),
                        op=ALU.add)
                    dot_sb = sb.tile([128, rsz], BF, tag="dot", bufs=3,
                                     name=f"dot{m2}_{roff}")
                    nc.scalar.activation(dot_sb[:], ps2[:], AF.Tanh,
                                         bias=tanhb_sb[:, m2:m2 + 1])
                    nc.tensor.matmul(scb[32 * ni:32 * ni + 1, 0:rsz],
                                     alpha_sb[:, m2:m2 + 1], dot_sb[:],
                                     start=(m2 == 0), stop=(m2 == AT - 1),
                                     tile_position=(0, 32 * ni))

            # ---- phase D: softmax over s (per b) ----------------------
            scores_sb = sb.tile([1, R], BF)
            for ni, (roff, rsz, bo, bt) in enumerate(RT):
                nc.scalar.activation(scores_sb[:, roff:roff + rsz],
                                     scb[32 * ni:32 * ni + 1, 0:rsz], AF.Exp)
            ssum = sb.tile([1, BC], F32)
            nc.vector.tensor_reduce(ssum[:],
                                    scores_sb.rearrange("p (b s) -> p b s", s=S),
                                    axis=mybir.AxisListType.X, op=ALU.add)
            rinv = sb.tile([1, BC, 1], F32)
            nc.vector.reciprocal(rinv[:, :, 0], ssum[:])
            w_bf = sb.tile([1, R], BF)
            nc.vector.tensor_tensor(
                out=w_bf.rearrange("p (b s) -> p b s", s=S),
                in0=scores_sb.rearrange("p (b s) -> p b s", s=S),
                in1=rinv.broadcast_to((1, BC, S)),
                op=ALU.mult)
            w_row = dram.tile([1, R], BF)
            nc.gpsimd.dma_start(out=w_row[:], in_=w_bf[:])
            w_exp = sb.tile([128, R], BF)
            nc.gpsimd.dma_start(out=w_exp[:], in_=w_row.broadcast_to((128, R)))

            # ---- phase E: att_res^T + 2 half AllGathers ---------------
            # ar_all[p, m, b]; DRAM halves laid out [p][kk*b] so both the
            # store and the post-gather reload use >=512B contiguous runs.
            ar_half = [sb.tile([128, AT // 2, BC], BF, name=f"ar_half{h}")
                       for h in range(2)]
            for m in range(AT):
                prodm = sb.tile([128, R], BF, tag="prod", bufs=1, name=f"prod{m}")
                nc.vector.tensor_tensor(out=prodm[:], in0=att_f[:, m, :],
                                        in1=w_exp[:], op=ALU.mult)
                arf = sb.tile([128, BC], F32, tag="arf", bufs=2, name=f"arf{m}")
                nc.vector.tensor_reduce(arf[:],
                                        prodm.rearrange("p (b s) -> p b s", s=S),
                                        axis=mybir.AxisListType.X, op=ALU.add)
                nc.scalar.activation(ar_half[m // 8][:, m % 8, :], arf[:], AF.Copy)
            ar_loc = [dram.tile([128, KT // 2 * BC], BF, name=f"ar_loc{h}")
                      for h in range(2)]
            ar_g = [dram.tile([NCORE * 128, KT // 2 * BC], BF,
                              addr_space="Shared", name=f"ar_g{h}")
                    for h in range(2)]
            arT = [sb.tile([128, KT // 2, NCORE, BC], BF, name=f"arT{h}")
                   for h in range(2)]
            for h in range(2):
                nc.gpsimd.dma_start(
                    out=ar_loc[h].rearrange("p (kk b) -> p kk b", b=BC),
                    in_=ar_half[h][:])
                nc.gpsimd.collective_compute(
                    "AllGather", ALU.bypass,
                    replica_groups=[list(range(NCORE))],
                    ins=[ar_loc[h].opt()], outs=[ar_g[h].opt()])
                for c in range(NCORE):
                    eng = nc.scalar if c % 2 == 0 else nc.sync
                    eng.dma_start(
                        out=arT[h][:, :, c, :],
                        in_=ar_g[h][c * 128:(c + 1) * 128, :].rearrange(
                            "p (kk b) -> p kk b", b=BC))

            # ---- phase F: LSTM (tensor-parallel over H chunk) ---------
            xt_sb = sb.tile([128, KT, B], BF)
            nc.sync.dma_start(out=xt_sb[:],
                              in_=embt_d.ap().rearrange("(k p) b -> p k b", p=128))
            nc.scalar.activation(xt_sb[:], xt_sb[:], AF.Relu)
            hprev_sb = sb.tile([128, KT, B], BF)
            nc.sync.dma_start(out=hprev_sb[:],
                              in_=hprevt_d.ap().rearrange("(k p) b -> p k b", p=128))
            cprev_sb = sb.tile([128, 2, HC], F32)
            nc.sync.dma_start(out=cprev_sb[:],
                              in_=cprev_d.ap().rearrange("(m p) h -> p m h", p=128))
            ones_sb = sb.tile([1, B], BF)
            nc.vector.memset(ones_sb[:], 1.0)
            i2hb_sb = sb.tile([1, 5 * HC], BF)
            nc.sync.dma_start(out=i2hb_sb[:], in_=i2h_d.ap()[H:H + 1, :])
            a2cb_sb = sb.tile([1, 2 * HC], BF)
            nc.sync.dma_start(out=a2cb_sb[:], in_=a2c_d.ap()[H:H + 1, :])
            i2h_v = i2h_d.ap()[0:H, :].rearrange("(k p) n -> p k n", p=128)
            h2h_v = h2h_d.ap().rearrange("(k p) n -> p k n", p=128)

            LT = [(0, 512), (512, 512), (1024, 256)]
            psl = {}
            sig_if, sig_o, intr_a = {}, {}, {}
            for (noff, nsz) in LT:
                for mb in range(2):
                    psl[(noff, mb)] = ps.tile([128, nsz], F32, tag="acc", bufs=7,
                                              name=f"ps_l{noff}_{mb}")
                for k in range(KT):
                    ri = sb.tile([128, nsz], BF, tag="lrhs", bufs=4,
                                 name=f"ri{noff}_{k}")
                    nc.sync.dma_start(out=ri[:], in_=i2h_v[:, k, noff:noff + nsz])
                    for mb in range(2):
                        nc.tensor.matmul(psl[(noff, mb)][:],
                                         xt_sb[:, k, mb * 128:(mb + 1) * 128],
                                         ri[:], start=(k == 0), stop=False)
                for k in range(KT):
                    rh = sb.tile([128, nsz], BF, tag="lrhs", bufs=4,
                                 name=f"rh{noff}_{k}")
                    nc.sync.dma_start(out=rh[:], in_=h2h_v[:, k, noff:noff + nsz])
                    for mb in range(2):
                        nc.tensor.matmul(psl[(noff, mb)][:],
                                         hprev_sb[:, k, mb * 128:(mb + 1) * 128],
                                         rh[:], start=False, stop=False)
                for mb in range(2):
                    nc.tensor.matmul(psl[(noff, mb)][:],
                                     ones_sb[:, mb * 128:(mb + 1) * 128],
                                     i2hb_sb[:, noff:noff + nsz],
                                     start=False, stop=(noff == 0))
                if noff >= 512:
                    # a2c contribution lands in in_tr cols [768:1280) of 5*HC
                    if noff == 512:
                        dsl, acs = slice(256, 512), slice(0, 256)
                    else:
                        dsl, acs = slice(0, 256), slice(256, 512)
                    for k in range(KT):
                        h, kk = divmod(k, KT // 2)
                        for mb in range(2):
                            nc.tensor.matmul(psl[(noff, mb)][:, dsl],
                                             arT[h][:, kk, mb * 4:(mb + 1) * 4, :],
                                             a2c_sb[:, k, acs],
                                             start=False, stop=False)
                    for mb in range(2):
                        nc.tensor.matmul(psl[(noff, mb)][:, dsl],
                                         ones_sb[:, mb * 128:(mb + 1) * 128],
                                         a2cb_sb[:, acs], start=False, stop=True)
                # drain
                for mb in range(2):
                    p_ = psl[(noff, mb)]
                    if noff == 0:
                        t_ = sb.tile([128, 512], F32, tag="sif", bufs=2,
                                     name=f"sif{mb}")
                        nc.scalar.activation(t_[:], p_[:], AF.Sigmoid)
                        sig_if[mb] = t_
                    elif noff == 512:
                        t_ = sb.tile([128, 256], F32, tag="sio", bufs=2,
                                     name=f"sio{mb}")
                        nc.scalar.activation(t_[:], p_[:, 0:256], AF.Sigmoid)
                        sig_o[mb] = t_
                        t2_ = sb.tile([128, 256], F32, tag="itra", bufs=2,
                                      name=f"itra{mb}")
                        nc.vector.tensor_copy(t2_[:], p_[:, 256:512])
                        intr_a[mb] = t2_

            for mb in range(2):
                bsl = slice(mb * 128, (mb + 1) * 128)
                in_tr = sb.tile([128, 256], F32, tag="intr", bufs=2, name=f"intr{mb}")
                nc.vector.tensor_tensor(out=in_tr[:],
                                        in0=intr_a[mb][:],
                                        in1=psl[(1024, mb)][:, 0:256], op=ALU.max)
                t_fc = sb.tile([128, 256], F32, tag="tfc", bufs=2, name=f"tfc{mb}")
                nc.vector.tensor_tensor(out=t_fc[:], in0=sig_if[mb][:, 256:512],
                                        in1=cprev_sb[:, mb, :], op=ALU.mult)
                t_ii = sb.tile([128, 256], F32, tag="tii", bufs=2, name=f"tii{mb}")
                nc.vector.tensor_tensor(out=t_ii[:], in0=sig_if[mb][:, 0:256],
                                        in1=in_tr[:], op=ALU.mult)
                next_c = sb.tile([128, 256], F32, tag="nc", bufs=2, name=f"nc{mb}")
                nc.vector.tensor_tensor(out=next_c[:], in0=t_fc[:], in1=t_ii[:],
                                        op=ALU.add)
                nc.gpsimd.dma_start(out=outc_d.ap()[bsl, :], in_=next_c[:])
                tanh_c = sb.tile([128, 256], F32, tag="tfc", bufs=2, name=f"tc{mb}")
                nc.scalar.activation(tanh_c[:], next_c[:], AF.Tanh)
                next_h = sb.tile([128, 256], F32, tag="tii", bufs=2, name=f"nh{mb}")
                nc.vector.tensor_tensor(out=next_h[:], in0=sig_o[mb][:],
                                        in1=tanh_c[:], op=ALU.mult)
                nc.gpsimd.dma_start(out=outh_d.ap()[bsl, :], in_=next_h[:])

    nc.compile()
    return nc


def _tile4(wt):
    """(2048,2048) [f,out] -> (16 m,128 p,16 k,128 j) contiguous bf16."""
    return np.ascontiguousarray(
        wt.reshape(KT, 128, AT, 128).transpose(2, 1, 0, 3)).astype(BF16)


def _bf(a):
    return np.ascontiguousarray(a).astype(BF16)


def prepare_inputs(V, v_g, state, fc_W, fc_b, att_W, att_b, emb, ctx_W, ctx_b,
                   a2c_W, a2c_b, i2h_W, i2h_b, h2h_W, h2h_b,
                   h2att_W, h2att_b, alpha_W, alpha_b, captions, time):
    t = int(np.asarray(time))
    V = np.asarray(V, np.float32)
    state = np.asarray(state, np.float32)
    if t == 0:
        state = np.zeros_like(state)
    h_prev = state[0, -1]
    c_prev = state[1, -1]
    it = np.asarray(captions, np.int64)[:, t]
    emb_it = np.asarray(emb, np.float32)[it]          # (B, H)

    attw4 = _tile4(np.asarray(att_W, np.float32).T)
    ctxw4 = _tile4(np.asarray(ctx_W, np.float32).T)
    h2attw4 = _tile4(np.asarray(h2att_W, np.float32).T)
    embt = _bf(emb_it.T)
    hprevt = _bf(h_prev.T)
    attb = np.ascontiguousarray(att_b, np.float32)
    tanhb = np.ascontiguousarray(np.asarray(ctx_b, np.float32)
                                 + np.asarray(h2att_b, np.float32))
    alpha = _bf(np.asarray(alpha_W, np.float32)[0])
    ib = np.asarray(i2h_b, np.float32) + np.asarray(h2h_b, np.float32)
    i2h_W = np.asarray(i2h_W, np.float32)
    h2h_W = np.asarray(h2h_W, np.float32)
    a2c_W = np.asarray(a2c_W, np.float32)
    a2c_b = np.asarray(a2c_b, np.float32)

    in_maps = []
    for c in range(NCORE):
        bs = slice(c * BC, (c + 1) * BC)
        hs = [slice(g * H + c * HC, g * H + (c + 1) * HC) for g in range(5)]
        i2h_blk = np.concatenate([i2h_W[s] for s in hs], 0)       # (1280, H)
        h2h_blk = np.concatenate([h2h_W[s] for s in hs], 0)
        ib_blk = np.concatenate([ib[s] for s in hs], 0)           # (1280,)
        a2c_blk = np.concatenate([a2c_W[s] for s in hs[:2]], 0)   # (512, H)
        a2cb_blk = np.concatenate([a2c_b[s] for s in hs[:2]], 0)
        vt = _bf(V[bs].transpose(2, 0, 1).reshape(H, R))
        in_maps.append({
            "vt": vt,
            "attw": attw4, "ctxw": ctxw4, "h2attw": h2attw4,
            "i2h": _bf(np.vstack([i2h_blk.T, ib_blk[None, :]])),
            "h2h": _bf(h2h_blk.T),
            "a2c": _bf(np.vstack([a2c_blk.T, a2cb_blk[None, :]])),
            "embt": embt, "hprevt": hprevt,
            "hprevown": np.ascontiguousarray(hprevt[:, bs]),
            "cprev": np.ascontiguousarray(c_prev[:, c * HC:(c + 1) * HC],
                                          ).astype(np.float32),
            "attb": attb, "tanhb": tanhb, "alpha": alpha,
        })
    return in_maps


def kernel(**inputs):
    in_maps = prepare_inputs(**inputs)
    if "nc" not in _NC_CACHE:
        _NC_CACHE["nc"] = build_nc()
    nc = _NC_CACHE["nc"]
    res = run_bass_kernel_spmd(nc, in_maps, core_ids=list(range(NCORE)))
    next_h = np.concatenate([res.results[c]["out_h"] for c in range(NCORE)], 1)
    next_c = np.concatenate([res.results[c]["out_c"] for c in range(NCORE)], 1)
    next_h = np.asarray(next_h, np.float32)
    next_c = np.asarray(next_c, np.float32)
    return next_h[:, None, :], next_h[None], next_c[None]
